# revision 14
# baseline (speedup 1.0000x reference)
"""Trainium2 Bass kernel for a 3-layer EdgeConv GNN classifier.

Sharding: nodes+edges partitioned across 8 cores by graph_id (4 graphs/core,
dst-owner partitioning). Per layer:
  t = h @ theta_w (per-core shard) -> AllGather -> full table in DRAM
  msg aggregation: agg[i] = max_{j in N_in(i)} t[j]  (gather + max)
  h' = (agg - t + p) computed as agg + h @ (phi_w - theta_w) + biases
Aggregation uses dma_gather (int16 idx, table chunk <= 32K rows) over 4 table
chunks, with per-chunk degree-sorted prefix "bucket" alignment so gathered
rows land directly on destination slots; a small second-level gather merges
the 4 chunk-ordered partials back into the common node order.
Readout: masked per-graph max + linear classifier, all on device.
"""
import sys
sys.path.insert(0, "/opt/trn_rl_repo")

import numpy as np
import concourse.bass as bass
import concourse.bacc as bacc
import concourse.tile as tile
import concourse.mybir as mybir
from concourse.masks import make_identity

# problem constants (hardcoded per contract)
N = 100000
E = 1600000
D = 64
L = 3
G = 32
CLS = 40
NCORES = 8
GPC = G // NCORES          # graphs per core
NCAP = 13824               # per-core node slot capacity (128*108, 4 quarters)
NTOT = NCORES * NCAP       # table rows
NCHUNK = 4                 # chunk q = quarter q of every core's region
GCH = 1024                 # max rows per dma_gather instruction
NQ = 4                     # SWDGE queues
NEG = np.float32(-1e30)


def _wrap16(arr):
    """[n] -> [128, n/16] int16 in dma_gather layout (16-wrap, x8 replicate)."""
    assert len(arr) % 16 == 0
    w = arr.astype(np.int16).reshape(-1, 16).T  # w[n%16, n//16]
    return np.tile(w, (8, 1)).copy()


def _prep(h, src, dst, graph_ids, theta_w, theta_b, phi_w, phi_b, cls_w, cls_b):
    """Host-side shard/index preparation. Returns (in_maps, plan)."""
    NB = NCAP // 128
    CR = NTOT // NCHUNK
    h = np.asarray(h, np.float32)
    src = np.asarray(src, np.int64)
    dst = np.asarray(dst, np.int64)
    graph_ids = np.asarray(graph_ids, np.int64)
    theta_w = np.asarray(theta_w, np.float32)
    theta_b = np.asarray(theta_b, np.float32)
    phi_w = np.asarray(phi_w, np.float32)
    phi_b = np.asarray(phi_b, np.float32)

    QR = NCAP // NCHUNK          # rows per quarter (incl. 128 sentinel slots)
    BQ = NB // NCHUNK            # blocks per quarter; block BQ-1 is reserved
    node_owner = graph_ids // GPC
    B = np.searchsorted(graph_ids, np.arange(0, G + GPC, GPC))
    nk = np.diff(B)
    reserved = np.concatenate([np.arange(q * QR + QR - 128, (q + 1) * QR)
                               for q in range(NCHUNK)])
    avail = np.setdiff1d(np.arange(NCAP), reserved)
    assert nk.max() <= len(avail), f"core node count {nk.max()} exceeds {len(avail)}"

    row_of = np.empty(N, np.int64)
    perms = []
    deg_tot = np.bincount(dst, minlength=N)
    for k in range(NCORES):
        nodes = np.arange(B[k], B[k + 1])
        degk = deg_tot[nodes]
        perm = np.argsort(-degk, kind="stable")
        slot = np.empty(len(nodes), np.int64)
        slot[perm] = avail[:len(nodes)]
        row_of[nodes] = k * NCAP + slot
        perms.append(perm)

    esrc_row = row_of[src]
    eowner = node_owner[dst]
    edst_rank = row_of[dst]  # k*NCAP + slot

    per_kc = [[None] * NCHUNK for _ in range(NCORES)]
    for k in range(NCORES):
        em = eowner == k
        es = esrc_row[em]
        ed = edst_rank[em] - k * NCAP
        ch = (es % NCAP) // QR   # quarter of the source row
        for c in range(NCHUNK):
            sel = ch == c
            # table rows are partition-major within each core's quarter:
            # slot s -> row k_src*QR + (s%128)*BQ_blocks + (s//128 - c*BQ_blocks)
            ssel = es[sel] % NCAP
            s_loc = ((es[sel] // NCAP) * QR + (ssel % 128) * BQ
                     + (ssel // 128 - c * BQ))
            d_rank = ed[sel]
            degc = np.bincount(d_rank, minlength=NCAP)
            orderc = np.argsort(-degc, kind="stable")
            invc = np.empty(NCAP, np.int64)
            invc[orderc] = np.arange(NCAP)
            eo = np.argsort(d_rank, kind="stable")
            ds = d_rank[eo]
            ss = s_loc[eo]
            if len(ds):
                starts = np.r_[0, np.flatnonzero(np.diff(ds)) + 1]
                run_id = np.zeros(len(ds), np.int64)
                run_id[starts[1:]] = 1
                run_id = np.cumsum(run_id)
                pos = np.arange(len(ds)) - starts[run_id]
            else:
                pos = np.zeros(0, np.int64)
            per_kc[k][c] = dict(degc=degc, invc=invc, ds=ds, ss=ss, pos=pos)

    plan_chunks = []
    for c in range(NCHUNK):
        maxdeg = max(int(per_kc[k][c]["degc"].max()) for k in range(NCORES))
        S = []
        for b in range(maxdeg):
            cnt = max(int(np.count_nonzero(per_kc[k][c]["degc"] > b))
                      for k in range(NCORES))
            S.append(128 * ((cnt + 127) // 128))
        if not S:
            S = [128]  # chunk with no edges anywhere: one all-sentinel bucket
        off = np.r_[0, np.cumsum(S)].astype(np.int64)
        plan_chunks.append(dict(S=S, off=off, rows=int(off[-1]),
                                aggc_rows=int(S[0]) + 128))

    chunk_col_off = np.r_[0, np.cumsum([p["rows"] // 16 for p in plan_chunks])]
    bidx_cols = int(chunk_col_off[-1])
    midx_cols = NCHUNK * (NCAP // 16)
    bidx = np.zeros((NCORES, 128, bidx_cols), np.int16)
    midx = np.zeros((NCORES, 128, midx_cols), np.int16)
    for k in range(NCORES):
        for c in range(NCHUNK):
            pc = plan_chunks[c]
            d = per_kc[k][c]
            # core-0 sentinel rows (slots x=BQ-1) in partition-major layout
            arr = (np.arange(pc["rows"]) % 128) * BQ + (BQ - 1)
            if len(d["ds"]):
                positions = pc["off"][d["pos"]] + d["invc"][d["ds"]]
                arr[positions] = d["ss"]
            bidx[k][:, chunk_col_off[c]:chunk_col_off[c + 1]] = _wrap16(arr)
            agb = pc["aggc_rows"] // 128
            s = d["invc"][np.arange(NCAP)]
            m = np.where(d["degc"] > 0,
                         (s % 128) * agb + s // 128,
                         (np.arange(NCAP) % 128) * agb + pc["S"][0] // 128)
            midx[k][:, c * (NCAP // 16):(c + 1) * (NCAP // 16)] = _wrap16(m)

    bplan = []
    for c in range(NCHUNK):
        pc = plan_chunks[c]
        for b, sz in enumerate(pc["S"]):
            done = 0
            while done < sz:
                m = min(GCH, sz - done)
                bplan.append((c, int(chunk_col_off[c] + (pc["off"][b] + done) // 16),
                              m, (int(pc["off"][b]) + done - int(pc["off"][b])) // 128
                              if False else done // 128, b == 0))
                done += m
    mplan = []  # quarter-major: (quarter, chunk, col, rows, h_blk, is_copy)
    for Q in range(NCHUNK):
        for c in range(NCHUNK):
            done = Q * QR
            end = (Q + 1) * QR
            while done < end:
                m = min(GCH, end - done)
                mplan.append((Q, c, c * (NCAP // 16) + done // 16, m,
                              done // 128, c == 0))
                done += m

    hsh = np.zeros((NCORES, 128, NB * D), np.float32)
    zmask = np.zeros((NCORES, 128, NB), np.float32)
    pmask = np.full((NCORES, 128, GPC * NB), NEG, np.float32)
    for k in range(NCORES):
        n = nk[k]
        slots = row_of[B[k]:B[k + 1]] - k * NCAP  # per original node order
        hp = np.zeros((NCAP, D), np.float32)
        hp[slots] = h[B[k]:B[k + 1]]
        hsh[k] = hp.reshape(NB, 128, D).transpose(1, 0, 2).reshape(128, NB * D)
        degs = np.zeros(NCAP, np.int64)
        degs[slots] = deg_tot[B[k]:B[k + 1]]
        assigned = np.zeros(NCAP, bool)
        assigned[slots] = True
        zm = (assigned & (degs > 0)).astype(np.float32)
        zmask[k] = zm.reshape(NB, 128).T
        gid = np.full(NCAP, -1, np.int64)
        gid[slots] = graph_ids[B[k]:B[k + 1]] - GPC * k
        for g in range(GPC):
            pm = np.where(gid == g, np.float32(0.0), NEG)
            pmask[k][:, g * NB:(g + 1) * NB] = pm.reshape(NB, 128).T

    w2 = np.zeros((2 * D, L * 128), np.float32)
    wb = np.zeros((128, L * D), np.float32)
    for l in range(L):
        w2[:D, l * 128:l * 128 + 64] = theta_w[l]
        w2[:D, l * 128 + 64:l * 128 + 128] = phi_w[l] - theta_w[l]
        wb[:, l * D:(l + 1) * D] = (theta_b[l] + phi_b[l])[None, :]
    w2[D:] = w2[:D]  # replicated for matmuls reading from base partition 64

    clsb = np.tile(np.asarray(cls_b, np.float32).reshape(1, CLS), (GPC, 1))
    in_maps = []
    for k in range(NCORES):
        in_maps.append({
            "hsh": hsh[k], "bidx": bidx[k], "midx": midx[k],
            "zmask": zmask[k], "pmask": pmask[k],
            "w2": w2, "wb": wb,
            "clsw": np.asarray(cls_w, np.float32),
            "clsb": clsb,
        })
    plan = dict(bidx_cols=bidx_cols, midx_cols=midx_cols, bplan=bplan,
                mplan=mplan, chunks=plan_chunks)
    return in_maps, plan


def _build_program(plan, reps=1, skip_bucket_dma=False, skip_merge_dma=False,
                   skip_collective=False):
    NB = NCAP // 128
    CR = NTOT // NCHUNK
    f32 = mybir.dt.float32
    i16 = mybir.dt.int16
    nc = bacc.Bacc("TRN2", target_bir_lowering=False, debug=False,
                   num_devices=NCORES, num_swdge_queues=NQ)
    hsh_t = nc.dram_tensor("hsh", [128, NB * D], f32, kind="ExternalInput")
    bidx_t = nc.dram_tensor("bidx", [128, plan["bidx_cols"]], i16,
                            kind="ExternalInput")
    midx_t = nc.dram_tensor("midx", [128, plan["midx_cols"]], i16,
                            kind="ExternalInput")
    zmask_t = nc.dram_tensor("zmask", [128, NB], f32, kind="ExternalInput")
    pmask_t = nc.dram_tensor("pmask", [128, GPC * NB], f32, kind="ExternalInput")
    w2_t = nc.dram_tensor("w2", [2 * D, L * 128], f32, kind="ExternalInput")
    wb_t = nc.dram_tensor("wb", [128, L * D], f32, kind="ExternalInput")
    clsw_t = nc.dram_tensor("clsw", [D, CLS], f32, kind="ExternalInput")
    clsb_t = nc.dram_tensor("clsb", [GPC, CLS], f32, kind="ExternalInput")
    out_t = nc.dram_tensor("out", [GPC, CLS], f32, kind="ExternalOutput")

    chunks = plan["chunks"]
    qcnt = [0]

    def next_q():
        q = qcnt[0] % NQ
        qcnt[0] += 1
        return q

    with tile.TileContext(nc) as tc:
        with tc.tile_pool(name="persist", bufs=1) as pp, \
             tc.tile_pool(name="stage", bufs=7) as sp, \
             tc.tile_pool(name="work", bufs=2) as wp, \
             tc.tile_pool(name="aggp", bufs=1) as ap_pool, \
             tc.tile_pool(name="psum", bufs=2, space="PSUM") as psp, \
             tc.tile_pool(name="dram", bufs=1, space="DRAM") as dp:
            h_sb = pp.tile([128, NB * D], f32)
            nc.sync.dma_start(h_sb[:], hsh_t[:])
            bidx_sb = pp.tile([128, plan["bidx_cols"]], i16)
            nc.sync.dma_start(bidx_sb[:], bidx_t[:])
            midx_sb = pp.tile([128, plan["midx_cols"]], i16)
            nc.sync.dma_start(midx_sb[:], midx_t[:])
            zmask_sb = pp.tile([128, NB], f32)
            nc.sync.dma_start(zmask_sb[:], zmask_t[:])
            pmask_sb = pp.tile([128, GPC * NB], f32)
            nc.sync.dma_start(pmask_sb[:], pmask_t[:])
            w2_sb = pp.tile([2 * D, L * 128], f32)
            nc.sync.dma_start(w2_sb[:], w2_t[:])
            wb_sb = pp.tile([128, L * D], f32)
            nc.sync.dma_start(wb_sb[:], wb_t[:])
            clsw_sb = pp.tile([D, CLS], f32)
            nc.sync.dma_start(clsw_sb[:], clsw_t[:])
            clsb_sb = pp.tile([GPC, CLS], f32)
            nc.sync.dma_start(clsb_sb[:], clsb_t[:])
            ident = pp.tile([128, 128], f32)
            make_identity(nc, ident[:])
            u_sb = pp.tile([128, NB * D], f32)

            BQl = NB // NCHUNK
            t_ownq = [dp.tile([128, BQl * D], f32, name=f"townq{q}")
                      for q in range(NCHUNK)]
            t_fulls_r = [[[dp.tile([CR, D], f32, addr_space="Shared",
                                   name=f"tfull{rep}_{l}_{q}")
                           for q in range(NCHUNK)] for l in range(L)]
                         for rep in range(reps)]
            ac_dram = [dp.tile([chunks[c]["aggc_rows"], D], f32, name=f"ac{c}")
                       for c in range(NCHUNK)]

            BQ = NB // NCHUNK
            QR = NCAP // NCHUNK
            for rep in range(reps):
              if rep > 0:
                nc.sync.dma_start(h_sb[:], hsh_t[:])
              t_fulls = t_fulls_r[rep]
              for l in range(L):
                t_full = t_fulls[l]
                # dense phase: t = h@tw -> DRAM; u = h@(pw-tw)+bias (SBUF)
                for b in range(NB):
                    if b % BQ == 0:
                        tq_sb = wp.tile([128, BQ * D], f32, tag="tq")
                        # reserved sentinel block (x=BQ-1) of this quarter
                        nc.vector.memset(
                            tq_sb[:, (BQ - 1) * D:BQ * D], NEG)
                    if b % 2 == 0:
                        hT_ps = psp.tile([128, 128], f32, tag="hT")
                        nc.tensor.transpose(
                            hT_ps[:], h_sb[:, b * D:(b + 2) * D], ident[:])
                        hT2 = wp.tile([128, 128], f32, tag="hTs")
                        nc.vector.tensor_copy(out=hT2[:], in_=hT_ps[:])
                    hT = hT2[(b % 2) * D:(b % 2 + 1) * D, :]
                    mm = psp.tile([128, 128], f32, tag="mm")
                    po = (b % 2) * D
                    nc.tensor.matmul(
                        mm[:], lhsT=hT,
                        rhs=w2_sb[po:po + D, l * 128:(l + 1) * 128],
                        start=True, stop=True)
                    x = b % BQ
                    if x < BQ - 1:
                        nc.vector.tensor_copy(
                            out=tq_sb[:, x * D:(x + 1) * D], in_=mm[:, 0:D])
                    nc.vector.tensor_tensor(
                        out=u_sb[:, b * D:(b + 1) * D], in0=mm[:, D:128],
                        in1=wb_sb[:, l * D:(l + 1) * D],
                        op=mybir.AluOpType.add)
                    if b % BQ == BQ - 1:
                        q = b // BQ
                        nc.sync.dma_start(t_ownq[q][:], tq_sb[:])
                        if not skip_collective:
                            nc.gpsimd.collective_compute(
                                "AllGather", mybir.AluOpType.bypass,
                                replica_groups=[list(range(NCORES))],
                                ins=[t_ownq[q][:].rearrange(
                                    "p (x d) -> (p x) d", d=D)],
                                outs=[t_full[q].opt()])

                # bucket gathers per chunk -> agg_c -> DRAM Ac
                aggc_tiles = {}
                cur_chunk = -1
                for (c, col, m, blk, is_copy) in plan["bplan"]:
                    if skip_bucket_dma:
                        continue
                    if c != cur_chunk:
                        cur_chunk = c
                        agb = chunks[c]["aggc_rows"] // 128
                        aggc = ap_pool.tile([128, agb * D], f32, tag="aggc",
                                            name=f"aggc{l}_{c}")
                        aggc_tiles[c] = aggc
                        nc.vector.memset(aggc[:, (agb - 1) * D:agb * D], NEG)
                    st = sp.tile([128, m // 128, D], f32, tag="st")
                    nc.gpsimd.dma_gather(
                        out_ap=st[:], in_ap=t_full[c][:],
                        idxs_ap=bidx_sb[:, col:col + m // 16],
                        num_idxs=m, num_idxs_reg=m, elem_size=D,
                        queue_num=next_q())
                    aggc = aggc_tiles[c]
                    dst_ap = aggc[:, blk * D:(blk + m // 128) * D]
                    st_flat = st[:].rearrange("p b d -> p (b d)")
                    if is_copy:
                        nc.vector.tensor_copy(out=dst_ap, in_=st_flat)
                    else:
                        nc.vector.tensor_tensor(out=dst_ap, in0=dst_ap, in1=st_flat,
                                                op=mybir.AluOpType.max)
                if not skip_bucket_dma:
                    for c in range(NCHUNK):
                        nc.sync.dma_start(
                            ac_dram[c][:].rearrange("(p x) d -> p (x d)", p=128),
                            aggc_tiles[c][:])

                # merge chunk partials into common order (into h_sb),
                # quarter-major so each h quarter finalizes early
                for (Q, c, col, m, blk, is_copy) in plan["mplan"]:
                    if not skip_merge_dma:
                        st = sp.tile([128, m // 128, D], f32, tag="st")
                        nc.gpsimd.dma_gather(
                            out_ap=st[:], in_ap=ac_dram[c][:],
                            idxs_ap=midx_sb[:, col:col + m // 16],
                            num_idxs=m, num_idxs_reg=m, elem_size=D,
                            queue_num=next_q())
                        dst_ap = h_sb[:, blk * D:(blk + m // 128) * D]
                        st_flat = st[:].rearrange("p b d -> p (b d)")
                        if is_copy:
                            nc.vector.tensor_copy(out=dst_ap, in_=st_flat)
                        else:
                            nc.vector.tensor_tensor(out=dst_ap, in0=dst_ap,
                                                    in1=st_flat,
                                                    op=mybir.AluOpType.max)
                    if c == NCHUNK - 1 and (blk + m // 128) == (Q + 1) * BQ:
                        # quarter complete: h_q = (agg_q + u_q) * zmask_q
                        hq = h_sb[:, Q * BQ * D:(Q + 1) * BQ * D]
                        nc.vector.tensor_tensor(
                            out=hq, in0=hq,
                            in1=u_sb[:, Q * BQ * D:(Q + 1) * BQ * D],
                            op=mybir.AluOpType.add)
                        nc.vector.tensor_tensor(
                            out=hq.rearrange("p (b d) -> p b d", d=D),
                            in0=hq.rearrange("p (b d) -> p b d", d=D),
                            in1=zmask_sb[:, Q * BQ:(Q + 1) * BQ, None]
                                .to_broadcast([128, BQ, D]),
                            op=mybir.AluOpType.mult)

            # readout: per-graph masked max -> classifier
            pooledT = pp.tile([D, GPC], f32)
            for g in range(GPC):
                tmp = wp.tile([128, NB * D], f32, tag="ptmp")
                nc.vector.tensor_tensor(
                    out=tmp[:].rearrange("p (b d) -> p b d", d=D),
                    in0=h_sb[:].rearrange("p (b d) -> p b d", d=D),
                    in1=pmask_sb[:, g * NB:(g + 1) * NB, None]
                        .to_broadcast([128, NB, D]),
                    op=mybir.AluOpType.add)
                red = wp.tile([128, D], f32, tag="red")
                nc.vector.reduce_max(
                    out=red[:].rearrange("p (d o) -> p d o", o=1),
                    in_=tmp[:].rearrange("p (b d) -> p d b", d=D),
                    axis=mybir.AxisListType.X)
                redT_ps = psp.tile([D, 128], f32, tag="redT")
                nc.tensor.transpose(redT_ps[:], red[:], ident[:])
                nc.vector.reduce_max(
                    out=pooledT[:, g:g + 1], in_=redT_ps[:],
                    axis=mybir.AxisListType.X)
            cls_ps = psp.tile([GPC, CLS], f32, tag="cls")
            nc.tensor.matmul(cls_ps[:], lhsT=pooledT[:], rhs=clsw_sb[:],
                             start=True, stop=True)
            out_sb = wp.tile([GPC, CLS], f32, tag="osb")
            nc.vector.tensor_tensor(out=out_sb[:], in0=cls_ps[:], in1=clsb_sb[:],
                                    op=mybir.AluOpType.add)
            nc.sync.dma_start(out_t[:], out_sb[:])

    nc.compile()
    return nc


_CACHE = {}


def kernel(**inputs):
    in_maps, plan = _prep(**inputs)
    key = tuple((t[0], t[1], t[2]) for t in plan["bplan"])
    if key not in _CACHE:
        _CACHE[key] = _build_program(plan)
    nc = _CACHE[key]
    from concourse.bass_utils import run_bass_kernel_spmd
    res = run_bass_kernel_spmd(nc, in_maps, core_ids=list(range(NCORES)))
    out = np.concatenate([res.results[k]["out"] for k in range(NCORES)], axis=0)
    return out.astype(np.float32)

